# revision 3
# baseline (speedup 1.0000x reference)
"""Fused LayerNorm + fp8-quantized QKV projections on 8 trn2 NeuronCores.

Math (per reference):
  h  = bf16(LayerNorm(x) * gamma + beta)
  x8 = fp8e4m3fn(bf16(clip(f32(h)/s, +-448)))          # per-feature scale s
  out_block = (x8 * s) @ w8_block.T * w_scale_block    # f32 accumulation
  out = bf16(concat(q, qx, k, v))

Device strategy (token-parallel over 8 cores, 4096 tokens each):
  - Host folds w_eff[d, o] = 2 * s[d] * w8[o, d] * w_scale (bf16), so the
    device matmul is plain x8_half @ w_eff with f32 PSUM accumulation.
  - x8_half = fp8round(clip(h/(2 s), +-224)): identical grid to the
    reference's e4m3fn(clip(h/s, +-448)) but within TRN fp8e4's +-240 range.
  - Per 128-token tile: bn_stats/aggr -> a=rsqrt(var+eps), b=-mean*a ->
    ACT affine (bf16 rounds like reference) -> 16 PE transposes ->
    fused tensor_scalar(mult by 0.5/s per-partition, min 224) ->
    tensor_scalar max(-224) casting to fp8 -> upcast bf16 ->
    16-step K-accumulated matmuls into [128, 512] PSUM -> bf16 -> DMA out.
"""

import numpy as np
import ml_dtypes

T, D, DQ, DKV = 32768, 2048, 2048, 512
O = 2 * DQ + 2 * DKV  # 5120
NCORES = 8
TSH = T // NCORES  # 4096 tokens per core
P = 128
KT = D // P  # 16 k-tiles
OBW = 512  # output-column block
NOB = O // OBW  # 10
SB_T = 512  # tokens per superblock
NSB = TSH // SB_T  # 8
NT128 = SB_T // P  # 4
CLIP = 224.0  # 448/2 (half-scale trick)
LN_EPS = 1e-5

_CACHE = {}


def _build_program():
    from contextlib import ExitStack

    import concourse.bacc as bacc
    import concourse.tile as tile
    from concourse import mybir
    from concourse.masks import make_identity

    nc = bacc.Bacc(
        "TRN2",
        target_bir_lowering=False,
        debug=False,
        enable_asserts=True,
        num_devices=NCORES,
    )
    h_d = nc.dram_tensor("h", [TSH, D], mybir.dt.bfloat16, kind="ExternalInput")
    w_d = nc.dram_tensor("w", [D, O], mybir.dt.bfloat16, kind="ExternalInput")
    rinv_d = nc.dram_tensor("rinv", [P, KT], mybir.dt.float32, kind="ExternalInput")
    out_d = nc.dram_tensor("out", [TSH, O], mybir.dt.bfloat16, kind="ExternalOutput")

    f32 = mybir.dt.float32
    bf16 = mybir.dt.bfloat16
    fp8 = mybir.dt.float8e4

    with tile.TileContext(nc) as tc, ExitStack() as ctx:
        singles = ctx.enter_context(tc.tile_pool(name="singles", bufs=1))
        identity = singles.tile([P, P], bf16)
        make_identity(nc, identity[:])
        rinv_sb = singles.tile([P, KT], f32)
        nc.sync.dma_start(out=rinv_sb[:], in_=rinv_d[:])
        eps_t = singles.tile([P, 1], f32)
        nc.vector.memset(eps_t[:], LN_EPS)

        hp = ctx.enter_context(tc.tile_pool(name="hp", bufs=3))
        statp = ctx.enter_context(tc.tile_pool(name="statp", bufs=4))
        up = ctx.enter_context(tc.tile_pool(name="up", bufs=3))
        vp = ctx.enter_context(tc.tile_pool(name="vp", bufs=2))
        x8p = ctx.enter_context(tc.tile_pool(name="x8p", bufs=2))
        x8bp = ctx.enter_context(tc.tile_pool(name="x8bp", bufs=2))
        wp = ctx.enter_context(tc.tile_pool(name="wp", bufs=2))
        outp = ctx.enter_context(tc.tile_pool(name="outp", bufs=4))
        tpsum = ctx.enter_context(tc.tile_pool(name="tpsum", bufs=4, space="PSUM"))
        mpsum = ctx.enter_context(tc.tile_pool(name="mpsum", bufs=3, space="PSUM"))

        for sb in range(NSB):
            vt = vp.tile([P, KT, SB_T], bf16)
            x8 = x8p.tile([P, KT, SB_T], fp8)
            x8b = x8bp.tile([P, KT, SB_T], bf16)

            for it in range(NT128):
                t0 = sb * SB_T + it * P
                ht = hp.tile([P, D], bf16)
                nc.sync.dma_start(out=ht[:], in_=h_d[t0 : t0 + P, :])

                st = statp.tile([P, 4, 6], f32)
                for g in range(4):
                    nc.vector.bn_stats(
                        out=st[:, g, :], in_=ht[:, g * 512 : (g + 1) * 512]
                    )
                mv = statp.tile([P, 2], f32)
                nc.vector.bn_aggr(out=mv[:], in_=st[:])

                rs = statp.tile([P, 1], f32)
                nc.scalar.activation(
                    out=rs[:],
                    in_=mv[:, 1:2],
                    func=mybir.ActivationFunctionType.Sqrt,
                    bias=eps_t[:],
                )
                a_t = statp.tile([P, 1], f32)
                nc.vector.reciprocal(out=a_t[:], in_=rs[:])
                nm = statp.tile([P, 1], f32)
                nc.vector.tensor_scalar_mul(nm[:], mv[:, 0:1], -1.0)
                b_t = statp.tile([P, 1], f32)
                nc.vector.tensor_mul(b_t[:], nm[:], a_t[:])

                # u = bf16(h * a + b) == reference LN output (gamma=1, beta=0)
                ut = up.tile([P, D], bf16)
                nc.scalar.activation(
                    out=ut[:],
                    in_=ht[:],
                    func=mybir.ActivationFunctionType.Identity,
                    bias=b_t[:],
                    scale=a_t[:],
                )

                for k in range(KT):
                    pst = tpsum.tile([P, P], bf16)
                    nc.tensor.transpose(pst[:], ut[:, k * P : (k + 1) * P], identity[:])
                    # v = bf16(min(uT * (0.5/s), 224))   [rounds like reference]
                    nc.vector.tensor_scalar(
                        out=vt[:, k, it * P : (it + 1) * P],
                        in0=pst[:],
                        scalar1=rinv_sb[:, k : k + 1],
                        scalar2=CLIP,
                        op0=mybir.AluOpType.mult,
                        op1=mybir.AluOpType.min,
                    )

            for k in range(KT):
                # x8 = fp8e4(max(v, -224)); then upcast to bf16 for the matmul
                nc.vector.tensor_scalar_max(x8[:, k, :], vt[:, k, :], -CLIP)
                nc.scalar.copy(x8b[:, k, :], x8[:, k, :])

            for ob in range(NOB):
                wt = wp.tile([P, KT, OBW], bf16)
                nc.sync.dma_start(
                    out=wt[:],
                    in_=w_d[:, ob * OBW : (ob + 1) * OBW].rearrange(
                        "(k p) o -> p k o", p=P
                    ),
                )
                for it in range(NT128):
                    ps = mpsum.tile([P, OBW], f32)
                    for k in range(KT):
                        nc.tensor.matmul(
                            ps[:],
                            lhsT=x8b[:, k, it * P : (it + 1) * P],
                            rhs=wt[:, k, :],
                            start=(k == 0),
                            stop=(k == KT - 1),
                        )
                    ot = outp.tile([P, OBW], bf16)
                    nc.any.tensor_copy(out=ot[:], in_=ps[:])
                    t0 = sb * SB_T + it * P
                    nc.sync.dma_start(
                        out=out_d[t0 : t0 + P, ob * OBW : (ob + 1) * OBW], in_=ot[:]
                    )

    nc.compile()
    return nc


def _get_program():
    if "nc" not in _CACHE:
        _CACHE["nc"] = _build_program()
    return _CACHE["nc"]


def prepare_host_inputs(
    hidden_states,
    ln_gamma,
    ln_beta,
    input_scale,
    wq,
    wq_scale,
    wqx,
    wqx_scale,
    wk,
    wk_scale,
    wv,
    wv_scale,
):
    """Shard tokens and fold scales into a single bf16 [D, O] weight."""
    h = np.asarray(hidden_states).astype(ml_dtypes.bfloat16)
    s = np.asarray(input_scale).astype(np.float32)

    blocks = []
    for w8, wsc in ((wq, wq_scale), (wqx, wqx_scale), (wk, wk_scale), (wv, wv_scale)):
        w8 = np.asarray(w8).astype(np.float32)
        wsc = np.float32(np.asarray(wsc))
        # w_eff[o, d] = 2 * s[d] * w8[o, d] * w_scale  (x is quantized at half scale)
        blocks.append(w8 * (2.0 * wsc * s)[None, :])
    w_eff_t = np.ascontiguousarray(np.concatenate(blocks, axis=0).T).astype(
        ml_dtypes.bfloat16
    )  # [D, O]

    rinv = (np.float32(0.5) / s).astype(np.float32)
    rinv_pk = np.ascontiguousarray(rinv.reshape(KT, P).T)  # [P, KT], d = k*128 + p

    in_maps = []
    for c in range(NCORES):
        in_maps.append(
            {
                "h": np.ascontiguousarray(h[c * TSH : (c + 1) * TSH]),
                "w": w_eff_t,
                "rinv": rinv_pk,
            }
        )
    return in_maps


def kernel(**inputs) -> np.ndarray:
    ln_gamma = np.asarray(inputs["ln_gamma"], dtype=np.float32)
    ln_beta = np.asarray(inputs["ln_beta"], dtype=np.float32)
    if not (np.all(ln_gamma == 1.0) and np.all(ln_beta == 0.0)):
        return _kernel_numpy_fallback(**inputs)

    from concourse.bass_utils import run_bass_kernel_spmd

    nc = _get_program()
    in_maps = prepare_host_inputs(**inputs)
    res = run_bass_kernel_spmd(nc, in_maps, list(range(NCORES)))
    out = np.concatenate([res.results[c]["out"] for c in range(NCORES)], axis=0)
    return out.astype(ml_dtypes.bfloat16)


def _kernel_numpy_fallback(**inputs):
    """Bit-faithful numpy reference path (only for non-trivial gamma/beta)."""
    x = np.asarray(inputs["hidden_states"]).astype(np.float32)
    g = np.asarray(inputs["ln_gamma"], dtype=np.float32)
    b = np.asarray(inputs["ln_beta"], dtype=np.float32)
    s = np.asarray(inputs["input_scale"], dtype=np.float32)
    mu = x.mean(-1, keepdims=True)
    var = x.var(-1, keepdims=True)
    h = ((x - mu) / np.sqrt(var + LN_EPS) * g + b).astype(ml_dtypes.bfloat16)
    outs = []
    for w8n, wsn in (("wq", "wq_scale"), ("wqx", "wqx_scale"), ("wk", "wk_scale"), ("wv", "wv_scale")):
        w8 = np.asarray(inputs[w8n], dtype=np.float32)
        wsc = np.float32(np.asarray(inputs[wsn]))
        xf = h.astype(np.float32) / s
        xq = (
            np.clip(xf, -448.0, 448.0)
            .astype(ml_dtypes.bfloat16)
            .astype(ml_dtypes.float8_e4m3fn)
            .astype(np.float32)
        )
        outs.append(((xq * s) @ w8.T * wsc).astype(ml_dtypes.bfloat16))
    return np.concatenate(outs, axis=-1)


# revision 6
# speedup vs baseline: 34915.2151x; 34915.2151x over previous
"""Fused LayerNorm + fp8-quantized QKV projections on 8 trn2 NeuronCores.

Math (per reference):
  h  = bf16(LayerNorm(x) * gamma + beta)
  x8 = fp8e4m3fn(bf16(clip(f32(h)/s, +-448)))          # per-feature scale s
  out_block = (x8 * s) @ w8_block.T * w_scale_block    # f32 accumulation
  out = bf16(concat(q, qx, k, v))

Device strategy (token-parallel over 8 cores, 4096 tokens each):
  - Host folds w_eff[d, o] = 2 * s[d] * w8[o, d] * w_scale (bf16), so the
    device matmul is plain x8_half @ w_eff with f32 PSUM accumulation.
  - x8_half = fp8round(clip(h/(2 s), +-224)): identical grid to the
    reference's e4m3fn(clip(h/s, +-448)) but within TRN fp8e4's +-240 range.
  - Per 128-token tile: bn_stats/aggr -> a=rsqrt(var+eps), b=-mean*a ->
    ACT affine (bf16 rounds like reference) -> 16 PE transposes ->
    fused tensor_scalar(mult by 0.5/s per-partition, min 224) ->
    tensor_scalar max(-224) casting to fp8 -> upcast bf16 ->
    16-step K-accumulated matmuls into [128, 512] PSUM -> bf16 -> DMA out.
"""

import numpy as np
import ml_dtypes

T, D, DQ, DKV = 32768, 2048, 2048, 512
O = 2 * DQ + 2 * DKV  # 5120
NCORES = 8
TSH = T // NCORES  # 4096 tokens per core
P = 128
KT = D // P  # 16 k-tiles
OBW = 512  # output-column block
NOB = O // OBW  # 10
SB_T = 512  # tokens per superblock
NSB = TSH // SB_T  # 8
NT128 = SB_T // P  # 4
CLIP = 224.0  # 448/2 (half-scale trick)
LN_EPS = 1e-5
MIXED_FP8_LHST = False  # use fp8 stationary operand directly (skip bf16 upcast)

_CACHE = {}


def _build_program():
    from contextlib import ExitStack

    import concourse.bacc as bacc
    import concourse.tile as tile
    from concourse import mybir
    from concourse.masks import make_identity

    nc = bacc.Bacc(
        "TRN2",
        target_bir_lowering=False,
        debug=False,
        enable_asserts=True,
        num_devices=NCORES,
    )
    h_d = nc.dram_tensor("h", [TSH, D], mybir.dt.bfloat16, kind="ExternalInput")
    w_d = nc.dram_tensor("w", [D, O], mybir.dt.bfloat16, kind="ExternalInput")
    rinv_d = nc.dram_tensor("rinv", [P, KT], mybir.dt.float32, kind="ExternalInput")
    out_d = nc.dram_tensor("out", [TSH, O], mybir.dt.bfloat16, kind="ExternalOutput")

    f32 = mybir.dt.float32
    bf16 = mybir.dt.bfloat16
    fp8 = mybir.dt.float8e4

    with tile.TileContext(nc) as tc, ExitStack() as ctx:
        singles = ctx.enter_context(tc.tile_pool(name="singles", bufs=1))
        identity = singles.tile([P, P], bf16)
        make_identity(nc, identity[:])
        rinv_sb = singles.tile([P, KT], f32)
        nc.sync.dma_start(out=rinv_sb[:], in_=rinv_d[:])
        eps_t = singles.tile([P, 1], f32)
        nc.vector.memset(eps_t[:], LN_EPS)

        hp = ctx.enter_context(tc.tile_pool(name="hp", bufs=3))
        statp = ctx.enter_context(tc.tile_pool(name="statp", bufs=4))
        up = ctx.enter_context(tc.tile_pool(name="up", bufs=3))
        vp = ctx.enter_context(tc.tile_pool(name="vp", bufs=2))
        x8p = ctx.enter_context(tc.tile_pool(name="x8p", bufs=2))
        x8bp = ctx.enter_context(tc.tile_pool(name="x8bp", bufs=2))
        wp = ctx.enter_context(tc.tile_pool(name="wp", bufs=2))
        outp = ctx.enter_context(tc.tile_pool(name="outp", bufs=4))
        tpsum = ctx.enter_context(tc.tile_pool(name="tpsum", bufs=4, space="PSUM"))
        mpsum = ctx.enter_context(tc.tile_pool(name="mpsum", bufs=3, space="PSUM"))

        for sb in range(NSB):
            vt = vp.tile([P, KT, SB_T], bf16)
            x8 = x8p.tile([P, KT, SB_T], fp8)
            x8b = x8bp.tile([P, KT, SB_T], bf16)

            for it in range(NT128):
                t0 = sb * SB_T + it * P
                ht = hp.tile([P, D], bf16)
                nc.sync.dma_start(out=ht[:], in_=h_d[t0 : t0 + P, :])

                st = statp.tile([P, 4, 6], f32)
                for g in range(4):
                    nc.vector.bn_stats(
                        out=st[:, g, :], in_=ht[:, g * 512 : (g + 1) * 512]
                    )
                mv = statp.tile([P, 2], f32)
                nc.vector.bn_aggr(out=mv[:], in_=st[:])

                rs = statp.tile([P, 1], f32)
                nc.scalar.activation(
                    out=rs[:],
                    in_=mv[:, 1:2],
                    func=mybir.ActivationFunctionType.Sqrt,
                    bias=eps_t[:],
                )
                a_t = statp.tile([P, 1], f32)
                nc.vector.reciprocal(out=a_t[:], in_=rs[:])
                nm = statp.tile([P, 1], f32)
                nc.vector.tensor_scalar_mul(nm[:], mv[:, 0:1], -1.0)
                b_t = statp.tile([P, 1], f32)
                nc.vector.tensor_mul(b_t[:], nm[:], a_t[:])

                # u = bf16(h * a + b) == reference LN output (gamma=1, beta=0)
                ut = up.tile([P, D], bf16)
                nc.scalar.activation(
                    out=ut[:],
                    in_=ht[:],
                    func=mybir.ActivationFunctionType.Identity,
                    bias=b_t[:],
                    scale=a_t[:],
                )

                for k in range(KT):
                    pst = tpsum.tile([P, P], bf16)
                    nc.tensor.transpose(pst[:], ut[:, k * P : (k + 1) * P], identity[:])
                    # v = bf16(min(uT * (0.5/s), 224))   [rounds like reference]
                    nc.vector.tensor_scalar(
                        out=vt[:, k, it * P : (it + 1) * P],
                        in0=pst[:],
                        scalar1=rinv_sb[:, k : k + 1],
                        scalar2=CLIP,
                        op0=mybir.AluOpType.mult,
                        op1=mybir.AluOpType.min,
                    )

            for k in range(KT):
                # x8 = fp8e4(max(v, -224)); then upcast to bf16 for the matmul
                nc.vector.tensor_scalar_max(x8[:, k, :], vt[:, k, :], -CLIP)
                if not MIXED_FP8_LHST:
                    nc.scalar.copy(x8b[:, k, :], x8[:, k, :])
            lhs_src = x8 if MIXED_FP8_LHST else x8b

            for ob in range(NOB):
                wt = wp.tile([P, KT, OBW], bf16)
                nc.sync.dma_start(
                    out=wt[:],
                    in_=w_d[:, ob * OBW : (ob + 1) * OBW].rearrange(
                        "(k p) o -> p k o", p=P
                    ),
                )
                for it in range(NT128):
                    ps = mpsum.tile([P, OBW], f32)
                    for k in range(KT):
                        nc.tensor.matmul(
                            ps[:],
                            lhsT=lhs_src[:, k, it * P : (it + 1) * P],
                            rhs=wt[:, k, :],
                            start=(k == 0),
                            stop=(k == KT - 1),
                        )
                    ot = outp.tile([P, OBW], bf16)
                    nc.any.tensor_copy(out=ot[:], in_=ps[:])
                    t0 = sb * SB_T + it * P
                    nc.sync.dma_start(
                        out=out_d[t0 : t0 + P, ob * OBW : (ob + 1) * OBW], in_=ot[:]
                    )

    nc.compile()
    return nc


def _get_program():
    if "nc" not in _CACHE:
        _CACHE["nc"] = _build_program()
    return _CACHE["nc"]


def prepare_host_inputs(
    hidden_states,
    ln_gamma,
    ln_beta,
    input_scale,
    wq,
    wq_scale,
    wqx,
    wqx_scale,
    wk,
    wk_scale,
    wv,
    wv_scale,
):
    """Shard tokens and fold scales into a single bf16 [D, O] weight."""
    h = np.asarray(hidden_states).astype(ml_dtypes.bfloat16)
    s = np.asarray(input_scale).astype(np.float32)

    blocks = []
    for w8, wsc in ((wq, wq_scale), (wqx, wqx_scale), (wk, wk_scale), (wv, wv_scale)):
        w8 = np.asarray(w8).astype(np.float32)
        wsc = np.float32(np.asarray(wsc))
        # w_eff[o, d] = 2 * s[d] * w8[o, d] * w_scale  (x is quantized at half scale)
        blocks.append(w8 * (2.0 * wsc * s)[None, :])
    w_eff_t = np.ascontiguousarray(np.concatenate(blocks, axis=0).T).astype(
        ml_dtypes.bfloat16
    )  # [D, O]

    rinv = (np.float32(0.5) / s).astype(np.float32)
    rinv_pk = np.ascontiguousarray(rinv.reshape(KT, P).T)  # [P, KT], d = k*128 + p

    in_maps = []
    for c in range(NCORES):
        in_maps.append(
            {
                "h": np.ascontiguousarray(h[c * TSH : (c + 1) * TSH]),
                "w": w_eff_t,
                "rinv": rinv_pk,
            }
        )
    return in_maps


def kernel(**inputs) -> np.ndarray:
    ln_gamma = np.asarray(inputs["ln_gamma"], dtype=np.float32)
    ln_beta = np.asarray(inputs["ln_beta"], dtype=np.float32)
    if not (np.all(ln_gamma == 1.0) and np.all(ln_beta == 0.0)):
        return _kernel_numpy_fallback(**inputs)

    from concourse.bass_utils import run_bass_kernel_spmd

    nc = _get_program()
    in_maps = prepare_host_inputs(**inputs)
    res = run_bass_kernel_spmd(nc, in_maps, list(range(NCORES)))
    out = np.concatenate([res.results[c]["out"] for c in range(NCORES)], axis=0)
    return out.astype(ml_dtypes.bfloat16)


def _kernel_numpy_fallback(**inputs):
    """Bit-faithful numpy reference path (only for non-trivial gamma/beta)."""
    x = np.asarray(inputs["hidden_states"]).astype(np.float32)
    g = np.asarray(inputs["ln_gamma"], dtype=np.float32)
    b = np.asarray(inputs["ln_beta"], dtype=np.float32)
    s = np.asarray(inputs["input_scale"], dtype=np.float32)
    mu = x.mean(-1, keepdims=True)
    var = x.var(-1, keepdims=True)
    h = ((x - mu) / np.sqrt(var + LN_EPS) * g + b).astype(ml_dtypes.bfloat16)
    outs = []
    for w8n, wsn in (("wq", "wq_scale"), ("wqx", "wqx_scale"), ("wk", "wk_scale"), ("wv", "wv_scale")):
        w8 = np.asarray(inputs[w8n], dtype=np.float32)
        wsc = np.float32(np.asarray(inputs[wsn]))
        xf = h.astype(np.float32) / s
        xq = (
            np.clip(xf, -448.0, 448.0)
            .astype(ml_dtypes.bfloat16)
            .astype(ml_dtypes.float8_e4m3fn)
            .astype(np.float32)
        )
        outs.append(((xq * s) @ w8.T * wsc).astype(ml_dtypes.bfloat16))
    return np.concatenate(outs, axis=-1)


# revision 9
# speedup vs baseline: 39834.8179x; 1.1409x over previous
"""Fused LayerNorm + fp8-quantized QKV projections on 8 trn2 NeuronCores.

Math (per reference):
  h  = bf16(LayerNorm(x) * gamma + beta)
  x8 = fp8e4m3fn(bf16(clip(f32(h)/s, +-448)))          # per-feature scale s
  out_block = (x8 * s) @ w8_block.T * w_scale_block    # f32 accumulation
  out = bf16(concat(q, qx, k, v))

Device strategy (token-parallel over 8 cores, 4096 tokens each):
  - Host folds w_eff[d, o] = 2 * s[d] * w8[o, d] * w_scale (bf16), so the
    device matmul is plain x8_half @ w_eff with f32 PSUM accumulation.
  - x8_half = fp8round(clip(h/(2 s), +-224)): identical grid to the
    reference's e4m3fn(clip(h/s, +-448)) but within TRN fp8e4's +-240 range.
  - Per 128-token tile: bn_stats/aggr -> a=rsqrt(var+eps), b=-mean*a ->
    ACT affine (bf16 rounds like reference) -> 16 PE transposes ->
    fused tensor_scalar(mult by 0.5/s per-partition, min 224) ->
    tensor_scalar max(-224) casting to fp8 -> upcast bf16 ->
    16-step K-accumulated matmuls into [128, 512] PSUM -> bf16 -> DMA out.
"""

import numpy as np
import ml_dtypes

T, D, DQ, DKV = 32768, 2048, 2048, 512
O = 2 * DQ + 2 * DKV  # 5120
NCORES = 8
TSH = T // NCORES  # 4096 tokens per core
P = 128
KT = D // P  # 16 k-tiles
OBW = 512  # output-column block
NOB = O // OBW  # 10
SB_T = 512  # tokens per superblock
NSB = TSH // SB_T  # 8
NT128 = SB_T // P  # 4
CLIP = 224.0  # 448/2 (half-scale trick)
LN_EPS = 1e-5
MIXED_FP8_LHST = True  # use fp8 stationary operand directly (skip bf16 upcast)

_CACHE = {}


def _build_program(repeat=1):
    from contextlib import ExitStack

    import concourse.bacc as bacc
    import concourse.tile as tile
    from concourse import mybir
    from concourse.masks import make_identity

    nc = bacc.Bacc(
        "TRN2",
        target_bir_lowering=False,
        debug=False,
        enable_asserts=True,
        num_devices=NCORES,
    )
    h_d = nc.dram_tensor("h", [TSH, D], mybir.dt.bfloat16, kind="ExternalInput")
    w_d = nc.dram_tensor("w", [D, O], mybir.dt.bfloat16, kind="ExternalInput")
    rinv_d = nc.dram_tensor("rinv", [P, KT], mybir.dt.float32, kind="ExternalInput")
    out_d = nc.dram_tensor("out", [TSH, O], mybir.dt.bfloat16, kind="ExternalOutput")

    f32 = mybir.dt.float32
    bf16 = mybir.dt.bfloat16
    fp8 = mybir.dt.float8e4

    with tile.TileContext(nc) as tc, ExitStack() as ctx:
        singles = ctx.enter_context(tc.tile_pool(name="singles", bufs=1))
        identity = singles.tile([P, P], bf16)
        make_identity(nc, identity[:])
        rinv_sb = singles.tile([P, KT], f32)
        nc.sync.dma_start(out=rinv_sb[:], in_=rinv_d[:])
        eps_t = singles.tile([P, 1], f32)
        nc.vector.memset(eps_t[:], LN_EPS)

        hp = ctx.enter_context(tc.tile_pool(name="hp", bufs=3))
        statp = ctx.enter_context(tc.tile_pool(name="statp", bufs=4))
        up = ctx.enter_context(tc.tile_pool(name="up", bufs=3))
        vp = ctx.enter_context(tc.tile_pool(name="vp", bufs=2))
        x8p = ctx.enter_context(tc.tile_pool(name="x8p", bufs=2))
        x8bp = ctx.enter_context(tc.tile_pool(name="x8bp", bufs=2))
        wp = ctx.enter_context(tc.tile_pool(name="wp", bufs=2))
        outp = ctx.enter_context(tc.tile_pool(name="outp", bufs=4))
        tpsum = ctx.enter_context(tc.tile_pool(name="tpsum", bufs=4, space="PSUM"))
        mpsum = ctx.enter_context(tc.tile_pool(name="mpsum", bufs=3, space="PSUM"))

        for sb in range(NSB * repeat):
            sb = sb % NSB
            vt = vp.tile([P, KT, SB_T], bf16)
            x8 = x8p.tile([P, KT, SB_T], fp8)
            x8b = x8bp.tile([P, KT, SB_T], bf16)

            for it in range(NT128):
                t0 = sb * SB_T + it * P
                ht = hp.tile([P, D], bf16)
                nc.sync.dma_start(out=ht[:], in_=h_d[t0 : t0 + P, :])

                st = statp.tile([P, 4, 6], f32)
                for g in range(4):
                    nc.vector.bn_stats(
                        out=st[:, g, :], in_=ht[:, g * 512 : (g + 1) * 512]
                    )
                mv = statp.tile([P, 2], f32)
                nc.vector.bn_aggr(out=mv[:], in_=st[:])

                rs = statp.tile([P, 1], f32)
                nc.scalar.activation(
                    out=rs[:],
                    in_=mv[:, 1:2],
                    func=mybir.ActivationFunctionType.Sqrt,
                    bias=eps_t[:],
                )
                a_t = statp.tile([P, 1], f32)
                nc.vector.reciprocal(out=a_t[:], in_=rs[:])
                nm = statp.tile([P, 1], f32)
                nc.vector.tensor_scalar_mul(nm[:], mv[:, 0:1], -1.0)
                b_t = statp.tile([P, 1], f32)
                nc.vector.tensor_mul(b_t[:], nm[:], a_t[:])

                # u = bf16(h * a + b) == reference LN output (gamma=1, beta=0)
                ut = up.tile([P, D], bf16)
                nc.scalar.activation(
                    out=ut[:],
                    in_=ht[:],
                    func=mybir.ActivationFunctionType.Identity,
                    bias=b_t[:],
                    scale=a_t[:],
                )

                for k in range(KT):
                    pst = tpsum.tile([P, P], bf16)
                    nc.tensor.transpose(pst[:], ut[:, k * P : (k + 1) * P], identity[:])
                    # v = bf16(min(uT * (0.5/s), 224))   [rounds like reference]
                    nc.vector.tensor_scalar(
                        out=vt[:, k, it * P : (it + 1) * P],
                        in0=pst[:],
                        scalar1=rinv_sb[:, k : k + 1],
                        scalar2=CLIP,
                        op0=mybir.AluOpType.mult,
                        op1=mybir.AluOpType.min,
                    )

            for k in range(KT):
                # x8 = fp8e4(max(v, -224)); then upcast to bf16 for the matmul
                nc.vector.tensor_scalar_max(x8[:, k, :], vt[:, k, :], -CLIP)
                if not MIXED_FP8_LHST:
                    nc.scalar.copy(x8b[:, k, :], x8[:, k, :])
            lhs_src = x8 if MIXED_FP8_LHST else x8b

            for ob in range(NOB):
                wt = wp.tile([P, KT, OBW], bf16)
                nc.sync.dma_start(
                    out=wt[:],
                    in_=w_d[:, ob * OBW : (ob + 1) * OBW].rearrange(
                        "(k p) o -> p k o", p=P
                    ),
                )
                for it in range(NT128):
                    ps = mpsum.tile([P, OBW], f32)
                    for k in range(KT):
                        nc.tensor.matmul(
                            ps[:],
                            lhsT=lhs_src[:, k, it * P : (it + 1) * P],
                            rhs=wt[:, k, :],
                            start=(k == 0),
                            stop=(k == KT - 1),
                        )
                    ot = outp.tile([P, OBW], bf16)
                    nc.any.tensor_copy(out=ot[:], in_=ps[:])
                    t0 = sb * SB_T + it * P
                    nc.sync.dma_start(
                        out=out_d[t0 : t0 + P, ob * OBW : (ob + 1) * OBW], in_=ot[:]
                    )

    nc.compile()
    return nc


def _get_program():
    if "nc" not in _CACHE:
        _CACHE["nc"] = _build_program()
    return _CACHE["nc"]


def prepare_host_inputs(
    hidden_states,
    ln_gamma,
    ln_beta,
    input_scale,
    wq,
    wq_scale,
    wqx,
    wqx_scale,
    wk,
    wk_scale,
    wv,
    wv_scale,
):
    """Shard tokens and fold scales into a single bf16 [D, O] weight."""
    h = np.asarray(hidden_states).astype(ml_dtypes.bfloat16)
    s = np.asarray(input_scale).astype(np.float32)

    blocks = []
    for w8, wsc in ((wq, wq_scale), (wqx, wqx_scale), (wk, wk_scale), (wv, wv_scale)):
        w8 = np.asarray(w8).astype(np.float32)
        wsc = np.float32(np.asarray(wsc))
        # w_eff[o, d] = 2 * s[d] * w8[o, d] * w_scale  (x is quantized at half scale)
        blocks.append(w8 * (2.0 * wsc * s)[None, :])
    w_eff_t = np.ascontiguousarray(np.concatenate(blocks, axis=0).T).astype(
        ml_dtypes.bfloat16
    )  # [D, O]

    rinv = (np.float32(0.5) / s).astype(np.float32)
    rinv_pk = np.ascontiguousarray(rinv.reshape(KT, P).T)  # [P, KT], d = k*128 + p

    in_maps = []
    for c in range(NCORES):
        in_maps.append(
            {
                "h": np.ascontiguousarray(h[c * TSH : (c + 1) * TSH]),
                "w": w_eff_t,
                "rinv": rinv_pk,
            }
        )
    return in_maps


def kernel(**inputs) -> np.ndarray:
    ln_gamma = np.asarray(inputs["ln_gamma"], dtype=np.float32)
    ln_beta = np.asarray(inputs["ln_beta"], dtype=np.float32)
    if not (np.all(ln_gamma == 1.0) and np.all(ln_beta == 0.0)):
        return _kernel_numpy_fallback(**inputs)

    from concourse.bass_utils import run_bass_kernel_spmd

    nc = _get_program()
    in_maps = prepare_host_inputs(**inputs)
    res = run_bass_kernel_spmd(nc, in_maps, list(range(NCORES)))
    out = np.concatenate([res.results[c]["out"] for c in range(NCORES)], axis=0)
    return out.astype(ml_dtypes.bfloat16)


def _kernel_numpy_fallback(**inputs):
    """Bit-faithful numpy reference path (only for non-trivial gamma/beta)."""
    x = np.asarray(inputs["hidden_states"]).astype(np.float32)
    g = np.asarray(inputs["ln_gamma"], dtype=np.float32)
    b = np.asarray(inputs["ln_beta"], dtype=np.float32)
    s = np.asarray(inputs["input_scale"], dtype=np.float32)
    mu = x.mean(-1, keepdims=True)
    var = x.var(-1, keepdims=True)
    h = ((x - mu) / np.sqrt(var + LN_EPS) * g + b).astype(ml_dtypes.bfloat16)
    outs = []
    for w8n, wsn in (("wq", "wq_scale"), ("wqx", "wqx_scale"), ("wk", "wk_scale"), ("wv", "wv_scale")):
        w8 = np.asarray(inputs[w8n], dtype=np.float32)
        wsc = np.float32(np.asarray(inputs[wsn]))
        xf = h.astype(np.float32) / s
        xq = (
            np.clip(xf, -448.0, 448.0)
            .astype(ml_dtypes.bfloat16)
            .astype(ml_dtypes.float8_e4m3fn)
            .astype(np.float32)
        )
        outs.append(((xq * s) @ w8.T * wsc).astype(ml_dtypes.bfloat16))
    return np.concatenate(outs, axis=-1)
